# revision 19
# baseline (speedup 1.0000x reference)
"""BatchSplitFF (moe_routing) — Bass/Tile kernel for 8 TRN2 NeuronCores.

kernel(**inputs) takes FULL unsharded inputs (x [4,2048,1024] f32,
controller [1024,32,4], f1 [1024,32,4,32], bias [32,4,32],
f2 [32,4,32,1024]) and returns the FULL output [4,2048,1024] f32.

Sharding: data-parallel over the 256 token-groups -> 32 groups (1024
tokens) per core; params replicated. No collectives.

Per-core pipeline:
  1. x -> SBUF; PE fp32 transposes -> xT (d on partitions).
  2. logits^T [es=128, tok=1024] = controller^T @ xT in fp32 on PE
     (fp32 PE matmul measured maxerr ~5e-7 vs fp64 -- routing-safe for
     the observed min top-2 gap of 2.9e-6).
  3. DVE: +tiebreak, per-group max over the 32 tokens, perm mask (bf16),
     argmax index per (es, group).
  4. SWDGE dma_gather (SBUF-source, transpose): gathers the selected
     token vectors as Dt [d%128, d//128, slot] in bf16 -- the dispatch,
     with zero PE/DVE cost.
  5. PE FF1 per (expert-quad, d-chunk): inner[f,g] accumulated in PSUM,
     ACT applies bias+ReLU -> bf16.
  6. PE FF2 per quad with diagonal tile packing -> intermediate rows
     [s*32+g, d] fp32 -> bf16, laid out for the un-permute.
  7. SBUF->SBUF DMA swizzle to [es, g, d]; PE scatter matmul per group
     (perm^T as stationary) accumulates slots back to token rows.
  8. Out rows -> HBM.
"""

import sys

sys.path.insert(0, "/opt/trn_rl_repo")

import numpy as np

DM = 1024
T = 32          # tokens per group == experts per set
ES = 128        # expert slots = 32 experts x 4 sets
F = 32          # expert hidden size
B, SEQ = 4, 2048
N_CORES = 8
TOK = 1024      # tokens per core
G = 32          # groups per core
NQ = 32         # expert quads (quad q = expert e=q, sets s=0..3)

_CACHE = {}


def _build_nc(debug_outputs=False):
    import concourse.bass as bass
    import concourse.tile as tile
    from concourse import bacc, mybir, masks
    from contextlib import ExitStack

    fp32 = mybir.dt.float32
    bf16 = mybir.dt.bfloat16
    i16 = mybir.dt.int16
    i32 = mybir.dt.int32
    AX = mybir.AxisListType
    OP = mybir.AluOpType
    ACT_RELU = mybir.ActivationFunctionType.Relu

    nc = bacc.Bacc("TRN2", target_bir_lowering=False, debug=False,
                   num_devices=N_CORES)

    xs = nc.dram_tensor("xs", [TOK, DM], fp32, kind="ExternalInput").ap()
    ctrl = nc.dram_tensor("ctrl", [DM, ES], fp32, kind="ExternalInput").ap()
    f1 = nc.dram_tensor("f1", [DM, ES * F], fp32, kind="ExternalInput").ap()
    f2 = nc.dram_tensor("f2", [ES * F, DM], fp32, kind="ExternalInput").ap()
    bias = nc.dram_tensor("bias", [ES * F], fp32, kind="ExternalInput").ap()
    out = nc.dram_tensor("out", [TOK, DM], fp32, kind="ExternalOutput").ap()
    dbg = {}
    if debug_outputs:
        dbg["logt"] = nc.dram_tensor("dbg_logt", [ES, TOK], fp32,
                                     kind="ExternalOutput").ap()
        dbg["sel"] = nc.dram_tensor("dbg_sel", [ES, G], fp32,
                                    kind="ExternalOutput").ap()
        dbg["idxg"] = nc.dram_tensor("dbg_idxg", [16, 256], i16,
                                     kind="ExternalOutput").ap()
        dbg["dt"] = nc.dram_tensor("dbg_dt", [128, 8 * 1024], bf16,
                                   kind="ExternalOutput").ap()
        dbg["inner"] = nc.dram_tensor("dbg_inner", [128, NQ * G], bf16,
                                      kind="ExternalOutput").ap()
        dbg["inter"] = nc.dram_tensor("dbg_inter", [128, NQ * DM], bf16,
                                      kind="ExternalOutput").ap()

    with tile.TileContext(nc) as tc, ExitStack() as ctx:
        # ---------------------------------------- whole-kernel pools
        const_pool = ctx.enter_context(tc.tile_pool(name="const", bufs=1))
        route_pool = ctx.enter_context(tc.tile_pool(name="route", bufs=1))
        xbf_pool = ctx.enter_context(tc.tile_pool(name="xbf", bufs=1))
        inner_pool = ctx.enter_context(tc.tile_pool(name="inner", bufs=1))
        out_pool = ctx.enter_context(tc.tile_pool(name="outp", bufs=1))

        # ------------------------------------------------- constants
        ident = const_pool.tile([128, 128], fp32)
        masks.make_identity(nc, ident[:])
        ident_bf = const_pool.tile([128, 128], bf16)
        masks.make_identity(nc, ident_bf[:])

        iota_i = const_pool.tile([128, T], i32)
        nc.gpsimd.iota(iota_i[:], pattern=[[1, T]], base=0,
                       channel_multiplier=0)
        iota_t = const_pool.tile([128, T], fp32)
        nc.vector.tensor_copy(iota_t[:], iota_i[:])
        tie_f = const_pool.tile([128, T], fp32)
        nc.vector.tensor_scalar_mul(tie_f[:], iota_t[:], float(1e-6 / 31.0))

        goff_i = const_pool.tile([128, G], i32)
        nc.gpsimd.iota(goff_i[:], pattern=[[T, G]], base=0,
                       channel_multiplier=0)
        goff = const_pool.tile([128, G], fp32)
        nc.vector.tensor_copy(goff[:], goff_i[:])

        bias_sb = const_pool.tile([128, NQ], fp32)  # [(s,f), e]
        nc.sync.dma_start(bias_sb[:], bias.rearrange("(e p) -> p e", p=128))

        ctrl_sb = const_pool.tile([128, 8 * ES], fp32)
        ctrl_v = ctrl_sb[:].rearrange("p (dc es) -> p dc es", dc=8)
        nc.sync.dma_start(ctrl_v, ctrl.rearrange("(dc p) es -> p dc es", p=128))

        # ------------------------------------------------- x load
        phase1 = ExitStack()
        xpool = phase1.enter_context(tc.tile_pool(name="x", bufs=1))
        xtpool = phase1.enter_context(tc.tile_pool(name="xt", bufs=1))
        ps_t = phase1.enter_context(tc.tile_pool(name="ps_t", bufs=3, space="PSUM"))
        ps_l = phase1.enter_context(tc.tile_pool(name="ps_l", bufs=2, space="PSUM"))
        ps_sel_pool = phase1.enter_context(
            tc.tile_pool(name="ps_sel", bufs=1, space="PSUM"))
        x_sb = xpool.tile([128, 8 * DM], fp32)  # [p, tb, d]; tok = tb*128+p
        x3 = x_sb[:].rearrange("p (tb d) -> p tb d", tb=8)
        nc.sync.dma_start(x3, xs.rearrange("(tb p) d -> p tb d", p=128))

        # x in bf16 for SBUF-source dma_gather; same layout as x_sb:
        # token t at partition t%128, rank t//128 (2048B per rank).
        x_bf = xbf_pool.tile([128, 8 * DM], bf16)
        nc.scalar.copy(x_bf[:], x_sb[:])

        # ------------------------------------------------- xT via PE
        xt_sb = xtpool.tile([128, 8 * TOK], fp32)  # [p, dc, tok]
        xt3 = xt_sb[:].rearrange("p (dc t) -> p dc t", dc=8)
        for tb in range(8):
            for dc in range(8):
                pst = ps_t.tile([128, 128], fp32, tag="pst")
                nc.tensor.transpose(pst[:], x3[:, tb, dc * 128:(dc + 1) * 128],
                                    ident[:])
                nc.vector.tensor_copy(xt3[:, dc, tb * 128:(tb + 1) * 128],
                                      pst[:])

        # ------------------------------------------------- logits (fp32)
        logt = route_pool.tile([128, TOK], fp32)  # [es, (g,t)] (+tie)
        logt3 = logt[:].rearrange("p (g t) -> p g t", t=T)
        tie_b = tie_f[:].rearrange("p (o t) -> p o t", o=1)  # [128, 1, T]
        for th in range(2):
            psl = ps_l.tile([128, 512], fp32, tag="psl")
            for dc in range(8):
                nc.tensor.matmul(
                    psl[:], ctrl_v[:, dc, :],
                    xt3[:, dc, th * 512:(th + 1) * 512],
                    start=(dc == 0), stop=(dc == 7))
            nc.vector.tensor_tensor(
                logt3[:, th * 16:(th + 1) * 16, :],
                psl[:].rearrange("p (g t) -> p g t", t=T),
                tie_b.broadcast_to([128, 16, T]),
                op=OP.add)

        gmax = route_pool.tile([128, G], fp32)
        nc.vector.tensor_reduce(gmax[:], logt3, axis=AX.X, op=OP.max)

        perm = route_pool.tile([128, TOK], bf16)  # [es, (g,t)] 0/1
        nc.vector.tensor_tensor(
            perm[:].rearrange("p (g t) -> p g t", t=T), logt3,
            gmax[:].rearrange("p g -> p g ()").broadcast_to([128, G, T]),
            op=OP.is_equal)

        # perm^T [tok, es] bf16 for the dispatch matmuls
        permT = route_pool.tile([128, 8 * ES], bf16)  # [tok%128, tb, es]
        permT_v = permT[:].rearrange("p (tb es) -> p tb es", tb=8)
        for tb in range(8):
            psp = ps_sel_pool.tile([128, 128], bf16, tag="psp")
            nc.tensor.transpose(psp[:], perm[:, tb * 128:(tb + 1) * 128],
                                ident_bf[:])
            nc.vector.tensor_copy(permT_v[:, tb, :], psp[:])
        phase1.close()  # x_sb / xt_sb / phase-1 PSUM dead past here

        if debug_outputs:
            nc.sync.dma_start(dbg["logt"], logt[:])

        # ---------------------------------------- dispatch gather + FF1
        inner_sb = inner_pool.tile([128, NQ * G], bf16)  # [(s,f), q, g]
        inner_v = inner_sb[:].rearrange("p (q g) -> p q g", q=NQ)

        # ---- dispatch: Dt[d%128, dc, es, g] via x-stationary matmuls
        phase2 = ExitStack()
        dt_pool = phase2.enter_context(tc.tile_pool(name="dt", bufs=1))
        w_pool = phase2.enter_context(tc.tile_pool(name="w", bufs=2))
        dspps = ExitStack()
        ps_dsp = dspps.enter_context(
            tc.tile_pool(name="ps_dsp", bufs=3, space="PSUM"))
        dt_sb = dt_pool.tile([128, 8 * ES * G], bf16)
        dt_v = dt_sb[:].rearrange("p (dc es g) -> p dc es g", dc=8, es=ES)
        for g in range(G):
            j, tb = g % 4, g // 4
            psd = ps_dsp.tile([128, DM], fp32, tag="psd")
            for dc in range(8):
                nc.tensor.matmul(
                    psd[:, dc * 128:(dc + 1) * 128],
                    x_bf[j * 32:(j + 1) * 32, tb * DM + dc * 128:
                         tb * DM + (dc + 1) * 128],
                    permT_v[j * 32:(j + 1) * 32, tb, :],
                    start=True, stop=True,
                    tile_position=(j * 32, 0))
            dst = dt_v[:, :, :, g].rearrange("p dc es -> p (dc es)")
            if g % 2 == 0:
                nc.vector.tensor_copy(dst, psd[:])
            else:
                nc.scalar.copy(dst, psd[:])

        dspps.close()

        # ---- FF1 per quad
        ps_ff1 = phase2.enter_context(
            tc.tile_pool(name="ps_ff1", bufs=3, space="PSUM"))
        for q in range(NQ):
            f1_q = w_pool.tile([128, 8 * 128], bf16, tag="f1")
            f1_qv = f1_q[:].rearrange("p (dc c) -> p dc c", dc=8)
            nc.gpsimd.dma_start(
                f1_qv,
                f1[:, q * 128:(q + 1) * 128]
                .rearrange("(dc p) c -> p dc c", p=128))
            psq = ps_ff1.tile([128, G], fp32, tag="ff1")
            for s in range(4):
                esl = q * 4 + s
                for dc in range(8):
                    nc.tensor.matmul(
                        psq[s * 32:(s + 1) * 32, :],
                        f1_qv[:, dc, s * 32:(s + 1) * 32],
                        dt_v[:, dc, esl, :],
                        start=(dc == 0), stop=(dc == 7),
                        tile_position=(0, s * 32))
            nc.scalar.activation(
                inner_v[:, q, :], psq[:],
                ACT_RELU, bias=bias_sb[:, q:q + 1])

        phase2.close()  # Dt / f1 slabs dead past here

        if debug_outputs:
            nc.sync.dma_start(dbg["inner"], inner_sb[:])

        # ------------------------------------------------- FF2
        phase3 = ExitStack()
        w2_pool = phase3.enter_context(tc.tile_pool(name="w2", bufs=3))
        inter_pool = phase3.enter_context(tc.tile_pool(name="inter", bufs=1))
        ff2ps = ExitStack()
        ps_ff2 = ff2ps.enter_context(
            tc.tile_pool(name="ps_ff2", bufs=2, space="PSUM"))
        inter_sb = inter_pool.tile([128, NQ * DM], bf16)  # [(s,g), q, d]
        inter_v = inter_sb[:].rearrange("p (q d) -> p q d", q=NQ)
        for q in range(NQ):
            psf = ps_ff2.tile([128, DM], fp32, tag="ff2")
            f2_sl = w2_pool.tile([128, DM], bf16, tag="f2")
            nc.gpsimd.dma_start(f2_sl[:], f2[q * 128:(q + 1) * 128, :])
            for s in range(4):
                for nh in range(2):
                    nc.tensor.matmul(
                        psf[s * 32:(s + 1) * 32, nh * 512:(nh + 1) * 512],
                        inner_v[s * 32:(s + 1) * 32, q, :],
                        f2_sl[s * 32:(s + 1) * 32, nh * 512:(nh + 1) * 512],
                        start=True, stop=True,
                        tile_position=(s * 32, s * 32))
            if q % 2 == 0:
                nc.vector.tensor_copy(inter_v[:, q, :], psf[:])
            else:
                nc.scalar.copy(inter_v[:, q, :], psf[:])

        if debug_outputs:
            nc.sync.dma_start(dbg["inter"], inter_sb[:])

        ff2ps.close()

        # ---------------- swizzle to [es, g, d] via DRAM bounce
        bpool = phase3.enter_context(
            tc.tile_pool(name="bounce", bufs=1, space="DRAM"))
        bounce = bpool.tile([128, G * DM], bf16)
        b_v = bounce[:].rearrange("es (g d) -> es g d", d=DM)
        for q in range(NQ):
            nc.sync.dma_start(b_v[4 * q:4 * q + 4, :, :], inter_v[:, q, :])
        swz = inter_pool.tile([128, G * DM], bf16)  # [es=(q,s), g, d]
        nc.sync.dma_start(swz[:], bounce[:])
        swz3 = swz[:].rearrange("p (g d) -> p g d", g=G)

        # -------------------------------------- scatter matmul + out
        ps_sc = phase3.enter_context(
            tc.tile_pool(name="ps_sc", bufs=2, space="PSUM"))
        out_sb = out_pool.tile([128, 8 * DM], fp32)  # [(j,t), gq, d]
        out_v = out_sb[:].rearrange("p (gq d) -> p gq d", gq=8)
        out_h = out.rearrange("(gq p) d -> p gq d", p=128)
        for gq in range(8):
            pso = ps_sc.tile([128, DM], fp32, tag="psc")
            for j in range(4):
                g = gq * 4 + j
                for nh in range(2):
                    nc.tensor.matmul(
                        pso[j * 32:(j + 1) * 32, nh * 512:(nh + 1) * 512],
                        perm[:, g * T:(g + 1) * T],
                        swz3[:, g, nh * 512:(nh + 1) * 512],
                        start=True, stop=True,
                        tile_position=(0, j * 32))
            if gq % 2 == 0:
                nc.vector.tensor_copy(out_v[:, gq, :], pso[:])
            else:
                nc.scalar.copy(out_v[:, gq, :], pso[:])
            nc.sync.dma_start(out_h[:, gq, :], out_v[:, gq, :])

        phase3.close()

    nc.compile()
    return nc


def _get_nc(debug_outputs=False):
    key = ("nc", debug_outputs)
    if key not in _CACHE:
        _CACHE[key] = _build_nc(debug_outputs)
    return _CACHE[key]


def _prep_inputs(x, controller, f1, bias, f2):
    x = np.ascontiguousarray(np.asarray(x, dtype=np.float32))
    ctrl = np.ascontiguousarray(
        np.asarray(controller, dtype=np.float32).reshape(DM, ES))
    f1m = np.ascontiguousarray(
        np.asarray(f1, dtype=np.float32).reshape(DM, ES * F))
    f2m = np.ascontiguousarray(
        np.asarray(f2, dtype=np.float32).reshape(ES * F, DM))
    bias1 = np.ascontiguousarray(
        np.asarray(bias, dtype=np.float32).reshape(ES * F))
    xt = x.reshape(N_CORES, TOK, DM)
    in_maps = []
    for c in range(N_CORES):
        in_maps.append({
            "xs": np.ascontiguousarray(xt[c]),
            "ctrl": ctrl, "f1": f1m, "f2": f2m, "bias": bias1,
        })
    return in_maps


def run_on_hw(x, controller, f1, bias, f2, debug_outputs=False, trace=False,
              trace_kwargs=None):
    from concourse.bass_utils import run_bass_kernel_spmd

    nc = _get_nc(debug_outputs)
    in_maps = _prep_inputs(x, controller, f1, bias, f2)
    res = run_bass_kernel_spmd(
        nc, in_maps, list(range(N_CORES)), trace=trace,
        **(trace_kwargs or {}))
    outs = [res.results[c]["out"] for c in range(N_CORES)]
    full = np.concatenate(outs, axis=0).reshape(B, SEQ, DM)
    return full.astype(np.float32), res


def kernel(x, controller, f1, bias, f2):
    out, _ = run_on_hw(x, controller, f1, bias, f2)
    return out
